# revision 20
# baseline (speedup 1.0000x reference)
"""Trainium2 Bass kernel for nn_CausalSelfAttention_2860448219236 (v2).

Reference semantics (B=2, S=2048, H=1024, NH=16, HD=64, WINDOW=512, NEG=-1e4):
  q/k/v = heads(hs @ W{q,k,v}.T + b)
  mask  = causal(j>i: NEG) + window(j >= i-512: NEG) + attention_mask
  out   = softmax(q k^T/8 + mask) v

Because NEG=-1e4 and softmax subtracts the row max, the f32 result equals a
*binary*-masked softmax over the allowed set
  A(i) = {j <= i}        for i <= 512  (whole row carries the same -1e4)
       = {j <= i-513}    for i >= 513  (recent-window entries underflow to 0)

Sharding: core c = (batch b = c//4) x (head group g = c%4, heads 4g..4g+3).
Fully data-parallel SPMD - one program, per-core input slices, no collectives.

v2 design (cost model: matmul time = out-free-size rows; DMA issue ~630ns
HWDGE hold each; DVE/ACT time = free-size * cycle):
  - everything bf16 on SBUF (halves DMA bytes + SBUF; same matmul rate)
  - merged input DMAs (one per section, not per k-tile)
  - scoresT[s,t] grid as before (QK packs 2 heads via row tile_position),
    exp on ACT with per-partition attn-mask bias, 0/1 diagonal masks on Pool
  - PV TRANSPOSED: out[t,d] = te(block)^T @ v -> 65-row matmuls (half cost);
    Z via 1-row matmuls against a shared ones column. Normalization becomes
    per-partition: reciprocal of Z cols + tensor_scalar_mul. No broadcast
    DMAs, no transposes; output DRAM layout is [t, d] so the host gather is
    a plain slice assignment.
  - staggered emission: QK(sb+1) is emitted before PV(sb); projections are
    spread between attention chunks as PE gap fillers.
  - t=512 (the one column whose window boundary is not block-aligned) is
    recomputed exactly in a small row-layout pass hidden under pair-1.
"""

import numpy as np

S = 2048
H = 1024
B = 2
NH = 16
HD = 64
SCALE = 0.125
SB = 128          # s block
TC = 512          # t chunk
NTC = S // TC     # 4
NSB = S // SB     # 16
NHC = 4           # heads per core
KTS = S - 512     # kT/v s-extent needed by the main grid (1536)
NVT = KTS // SB   # 12 v tiles
NK = H // SB      # 8 k-tiles
WK = 768          # per-k-tile w cols: q0|k0|q1|k1|v(256)

_CACHE = {}


def _alive_sbs(tci):
    if tci == 0:
        return list(range(4))
    return list(range(min(4 * tci, NVT)))


def _x_lo(sb, tci):
    first_tb = sb if tci == 0 else sb + 4
    return max(0, SB * (first_tb - 4 * tci))


def _sb_last(tb, tci, nsbs):
    # last sb contributing to t-block tb of chunk tci
    if tci == 0:
        return tb
    return min(nsbs - 1, 4 * tci - 4 + tb)


def _diag_actions(sb, tci):
    """[(block_in_chunk, mask_idx, col_off)]; mask 0 = p<=x, 1 = p<=x-1."""
    acts = []
    for tb in range(4 * tci, 4 * tci + 4):
        if tb <= 3 and tb == sb:
            acts.append((tb - 4 * tci, 0, 0))
        if tb >= 4 and tb - 4 == sb:
            # at t0=512 (tci==1, block 0) leave col 0 unmasked: that column
            # (t=512) is recomputed exactly by the special pass, and masking
            # it fully would make Z=0 -> div noise.
            col_off = 1 if (tci == 1 and tb == 4) else 0
            acts.append((tb - 4 * tci, 1, col_off))
    return acts


def _build_program(with_bias=False, with_attc=False):
    import concourse.bass as bass_mod
    import concourse.bacc as bacc
    import concourse.mybir as mybir
    from concourse.tile import TileContext

    F32 = mybir.dt.float32
    BF16 = mybir.dt.bfloat16
    EXP = mybir.ActivationFunctionType.Exp

    nc = bacc.Bacc("TRN2", target_bir_lowering=False, debug=False)

    hst_d = nc.dram_tensor("hst", [SB, NK * S], BF16, kind="ExternalInput")
    w_d = nc.dram_tensor("w", [SB, NK * WK], BF16, kind="ExternalInput")
    attc_d = nc.dram_tensor("attc", [SB, NSB], F32, kind="ExternalInput")
    masks_d = nc.dram_tensor("masks", [SB, 2 * SB + 1], BF16,
                             kind="ExternalInput")
    attr_d = nc.dram_tensor("attr", [1, 513], F32, kind="ExternalInput")
    if with_attc:
        attcs_d = nc.dram_tensor("attcs", [SB, NSB], F32,
                                 kind="ExternalInput")
    if with_bias:
        hst9_d = nc.dram_tensor("hst9", [1, S], BF16, kind="ExternalInput")
        w9_d = nc.dram_tensor("w9", [1, WK], BF16, kind="ExternalInput")
    out_d = nc.dram_tensor("out", [S, NHC * HD], F32, kind="ExternalOutput")

    NKA = NK + (1 if with_bias else 0)

    with TileContext(nc) as tc:
        with tc.tile_pool(name="stat", bufs=1) as stat:
            hst = stat.tile([SB, NK * S], BF16, tag="hst", name="hst")
            wt = stat.tile([SB, NK * WK], BF16, tag="wt", name="wt")
            qt = [stat.tile([SB, S], BF16, tag=f"qt{e}", name=f"qt{e}")
                  for e in range(2)]
            kt = [stat.tile([SB, KTS], BF16, tag=f"kt{e}", name=f"kt{e}")
                  for e in range(2)]
            vt = [stat.tile([SB, NHC * (HD + 1)], BF16, tag=f"vt{i}", name=f"vt{i}")
                  for i in range(NVT)]
            attc = stat.tile([SB, NSB], F32, tag="attc", name="attc")
            masks = stat.tile([SB, 2 * SB + 1], BF16, tag="masks",
                              name="masks")
            attr_t = stat.tile([1, 513], F32, tag="attr", name="attr")
            # special pass (t=512): erT[s, 4g+sb] = probs column-major;
            # cols 16..19 (partition 0 only) hold the j=512 tail per g.
            erT = stat.tile([SB, 20], BF16, tag="erT", name="erT")
            rz4 = stat.tile([1, NHC], F32, tag="rz4", name="rz4")
            svn = stat.tile([1, NHC * HD], F32, tag="svn", name="svn")
            if with_attc:
                attcs = stat.tile([SB, NSB], F32, tag="attcs", name="attcs")
            if with_bias:
                hst9 = stat.tile([1, S], BF16, tag="hst9", name="hst9")
                w9 = stat.tile([1, WK], BF16, tag="w9", name="w9")

            for i in range(NVT):
                ocols = bass_mod.AP(
                    tensor=vt[i][:].tensor, offset=HD,
                    ap=[[NHC * (HD + 1), SB], [HD + 1, NHC], [1, 1]],
                )
                nc.vector.memset(ocols, 1.0)

            # --- merged, section-ordered input DMA (SP queue) -------------
            def dma_wqk(k0, k1, c0, c1):
                src = bass_mod.AP(
                    tensor=w_d.ap().tensor, offset=k0 * WK + c0,
                    ap=[[NK * WK, SB], [WK, k1 - k0], [1, c1 - c0]],
                )
                dst = bass_mod.AP(
                    tensor=wt[:].tensor, offset=k0 * WK + c0,
                    ap=[[NK * WK, SB], [WK, k1 - k0], [1, c1 - c0]],
                )
                nc.sync.dma_start(out=dst, in_=src)

            def dma_hst(cc, k0, k1):
                src = bass_mod.AP(
                    tensor=hst_d.ap().tensor, offset=k0 * S + cc * TC,
                    ap=[[NK * S, SB], [S, k1 - k0], [1, TC]],
                )
                dst = bass_mod.AP(
                    tensor=hst[:].tensor, offset=k0 * S + cc * TC,
                    ap=[[NK * S, SB], [S, k1 - k0], [1, TC]],
                )
                nc.sync.dma_start(out=dst, in_=src)

            def dma_wv(k0, k1):
                src = bass_mod.AP(
                    tensor=w_d.ap().tensor, offset=k0 * WK + 512,
                    ap=[[NK * WK, SB], [WK, k1 - k0], [1, 256]],
                )
                dst = bass_mod.AP(
                    tensor=wt[:].tensor, offset=k0 * WK + 512,
                    ap=[[NK * WK, SB], [WK, k1 - k0], [1, 256]],
                )
                nc.sync.dma_start(out=dst, in_=src)

            for kk, ke in ((0, 2), (2, 4), (4, 6), (6, 8)):
                dma_wqk(kk, ke, 0, 256)
                dma_hst(0, kk, ke)
                dma_wv(kk, ke)
            nc.sync.dma_start(out=attc[:], in_=attc_d[:])
            nc.sync.dma_start(out=masks[:], in_=masks_d[:])
            nc.sync.dma_start(out=attr_t[:], in_=attr_d[:])
            if with_attc:
                nc.sync.dma_start(out=attcs[:], in_=attcs_d[:])
            if with_bias:
                nc.sync.dma_start(out=hst9[:], in_=hst9_d[:])
                nc.sync.dma_start(out=w9[:], in_=w9_d[:])
            dma_hst(1, 0, 2)
            dma_hst(1, 2, 4)
            dma_hst(1, 4, 6)
            dma_hst(1, 6, NK)
            dma_wqk(0, NK, 256, 512)
            dma_hst(2, 0, 4)
            dma_hst(2, 4, NK)
            dma_hst(3, 0, 4)
            dma_hst(3, 4, NK)

            with (
                tc.tile_pool(name="mmps", bufs=2, space="PSUM") as mmps,
                tc.tile_pool(name="ppps", bufs=2, space="PSUM") as ppps,
                tc.tile_pool(name="pvps", bufs=2, space="PSUM") as pvps,
                tc.tile_pool(name="epool", bufs=56) as epool,
                tc.tile_pool(name="rpool", bufs=3) as rpool,
                tc.tile_pool(name="opool", bufs=3) as opool,
            ):

                def thunks_q(e, tcc):
                    box = {}
                    def mk(k):
                        def f():
                            if k == 0:
                                box["pp"] = ppps.tile([SB, TC], F32,
                                                      tag="pp", name="pp")
                            nc.tensor.matmul(
                                box["pp"][:],
                                wt[:, k * WK + 256 * e :
                                   k * WK + 256 * e + SB],
                                hst[:, k * S + tcc * TC :
                                    k * S + (tcc + 1) * TC],
                                start=(k == 0),
                                stop=(k == NKA - 1),
                            )
                            if with_bias and k == NK - 1:
                                nc.tensor.matmul(
                                    box["pp"][:],
                                    w9[:, 256 * e : 256 * e + SB],
                                    hst9[:, tcc * TC : (tcc + 1) * TC],
                                    start=False, stop=True,
                                )
                        return f
                    def cp():
                        nc.vector.tensor_copy(
                            qt[e][:, tcc * TC : (tcc + 1) * TC], box["pp"][:]
                        )
                    return [mk(k) for k in range(NK)] + [cp]

                def thunks_k(e, scc):
                    box = {}
                    off0 = 256 * e + SB
                    def mk(k):
                        def f():
                            if k == 0:
                                box["pp"] = ppps.tile([SB, TC], F32,
                                                      tag="pp", name="pp")
                            nc.tensor.matmul(
                                box["pp"][:],
                                wt[:, k * WK + off0 : k * WK + off0 + SB],
                                hst[:, k * S + scc * TC :
                                    k * S + (scc + 1) * TC],
                                start=(k == 0),
                                stop=(k == NKA - 1),
                            )
                            if with_bias and k == NK - 1:
                                nc.tensor.matmul(
                                    box["pp"][:],
                                    w9[:, off0 : off0 + SB],
                                    hst9[:, scc * TC : (scc + 1) * TC],
                                    start=False, stop=True,
                                )
                        return f
                    def cp():
                        nc.vector.tensor_copy(
                            kt[e][:, scc * TC : (scc + 1) * TC], box["pp"][:]
                        )
                    return [mk(k) for k in range(NK)] + [cp]

                def thunks_v(sb):
                    box = {}
                    def mk(k):
                        def f():
                            if k == 0:
                                box["pp"] = ppps.tile([SB, TC], F32,
                                                      tag="pp", name="pp")
                            nc.tensor.matmul(
                                box["pp"][:, 0:256],
                                hst[:, k * S + sb * SB :
                                    k * S + (sb + 1) * SB],
                                wt[:, k * WK + 512 : (k + 1) * WK],
                                start=(k == 0),
                                stop=(k == NKA - 1),
                            )
                            if with_bias and k == NK - 1:
                                nc.tensor.matmul(
                                    box["pp"][:, 0:256],
                                    hst9[:, sb * SB : (sb + 1) * SB],
                                    w9[:, 512:WK],
                                    start=False, stop=True,
                                )
                        return f
                    def cp():
                        vdst = bass_mod.AP(
                            tensor=vt[sb][:].tensor, offset=0,
                            ap=[[NHC * (HD + 1), SB], [HD + 1, NHC],
                                [1, HD]],
                        )
                        nc.vector.tensor_copy(vdst, box["pp"][:, 0:256])
                    return [mk(k) for k in range(NK)] + [cp]

                def emit_proj(ths):
                    for f in ths:
                        f()

                def emit_pv_group(pair, pvh, tes, tb, tci, nsbs):
                    # one t-block accumulation group; groups sharing a PSUM
                    # bank must run strictly sequentially (a start=True marks
                    # the whole 2KB bank pending-zero, so an interleaved
                    # foreign start would turn accumulates into overwrites)
                    last = _sb_last(tb, tci, nsbs)
                    for h2 in range(2):
                        for sb2 in range(last + 1):
                            teb = tes[sb2][:, h2 * TC + tb * SB :
                                           h2 * TC + (tb + 1) * SB]
                            nc.tensor.matmul(
                                pvh[h2][:, 65 * tb : 65 * tb + HD + 1],
                                teb,
                                vt[sb2][:, (2 * pair + h2) * (HD + 1) :
                                        (2 * pair + h2 + 1) * (HD + 1)],
                                start=(sb2 == 0),
                                stop=(sb2 == last),
                                skip_group_check=True,
                            )

                def emit_norm(pair, tci, pvh, tb0, tb1, act_dma=False,
                              dma_eng=None):
                    # per-partition Z -> reciprocal + free-broadcast mul
                    ntb = tb1 - tb0
                    r = rpool.tile([SB, 8], F32, tag="r", name="r")
                    for h2 in range(2):
                        zsrc = bass_mod.AP(
                            tensor=pvh[h2][:].tensor, offset=65 * tb0 + HD,
                            ap=[[260, SB], [65, ntb], [1, 1]],
                        )
                        nc.vector.reciprocal(
                            r[:, 4 * h2 : 4 * h2 + ntb], zsrc)
                    osb = opool.tile([SB, SB * ntb], F32, tag="osb",
                                     name="osb")
                    for h2 in range(2):
                        odst = bass_mod.AP(
                            tensor=osb[:].tensor, offset=h2 * HD,
                            ap=[[SB * ntb, SB], [SB, ntb], [1, HD]],
                        )
                        psrc = bass_mod.AP(
                            tensor=pvh[h2][:].tensor, offset=65 * tb0,
                            ap=[[260, SB], [65, ntb], [1, HD]],
                        )
                        rsrc = bass_mod.AP(
                            tensor=r[:].tensor, offset=4 * h2,
                            ap=[[8, SB], [1, ntb], [0, HD]],
                        )
                        nc.vector.tensor_mul(odst, psrc, rsrc)
                    skip512 = (tci == 1 and tb0 == 0)
                    dst = bass_mod.AP(
                        tensor=out_d.ap().tensor,
                        offset=(tci * TC + tb0 * SB + skip512) * 256
                        + 128 * pair,
                        ap=[[256, SB - skip512], [SB * 256, 1], [1, SB]],
                    )
                    src = bass_mod.AP(
                        tensor=osb[:].tensor, offset=skip512 * SB * ntb,
                        ap=[[SB * ntb, SB - skip512], [SB, 1], [1, SB]],
                    )
                    (dma_eng or (nc.scalar if act_dma else nc.sync)
                     ).dma_start(out=dst, in_=src)
                    if ntb > 1:
                        dst2 = bass_mod.AP(
                            tensor=out_d.ap().tensor,
                            offset=(tci * TC + (tb0 + 1) * SB) * 256
                            + 128 * pair,
                            ap=[[256, SB], [SB * 256, ntb - 1], [1, SB]],
                        )
                        src2 = bass_mod.AP(
                            tensor=osb[:].tensor, offset=SB,
                            ap=[[SB * ntb, SB], [SB, ntb - 1], [1, SB]],
                        )
                        (dma_eng or (nc.scalar if act_dma else nc.sync)
                         ).dma_start(out=dst2, in_=src2)

                def emit_attn(pair, tci, feed=(), tail_split=False,
                              alt_pqk=False, defer_pv=False):
                    sbs = _alive_sbs(tci)
                    n = len(sbs)
                    feed = list(feed)
                    rate = -(-len(feed) // n) if feed else 0
                    stag = 1 if tail_split else 2
                    if not defer_pv:
                        pvh = [pvps.tile([SB, 260], F32, tag="pv",
                                         name=f"pv{h2}") for h2 in range(2)]
                    last_i = [_sb_last(tb, tci, n) for tb in range(4)]
                    tes = []
                    emitted = 0
                    for i, sb in enumerate(sbs):
                        xlo = _x_lo(sb, tci)
                        if alt_pqk and i % 2 == 1:
                            pq2 = [ppps.tile([SB, TC], F32, tag="pp",
                                             name="pqh") for _ in range(2)]
                            halves = [pq2[h][:, xlo:TC] for h in range(2)]
                        else:
                            pqk = mmps.tile([SB, 2 * TC], F32, tag="mm",
                                            name="pqk")
                            halves = [pqk[:, h * TC + xlo : (h + 1) * TC]
                                      for h in range(2)]
                        for h2 in range(2):
                            nc.tensor.matmul(
                                halves[h2],
                                kt[pair][h2 * HD : (h2 + 1) * HD,
                                         sb * SB : (sb + 1) * SB],
                                qt[pair][h2 * HD : (h2 + 1) * HD,
                                         tci * TC + xlo : (tci + 1) * TC],
                                start=True,
                                stop=True,
                                tile_position=(h2 * HD, 0),
                            )
                        te = epool.tile([SB, 2 * TC], BF16, tag="te",
                                        name="te")
                        te3 = te[:].rearrange("p (b n) -> p b n", b=2)
                        if alt_pqk and i % 2 == 1:
                            for h2 in range(2):
                                nc.scalar.activation(
                                    te[:, h2 * TC + xlo : (h2 + 1) * TC],
                                    halves[h2],
                                    EXP,
                                    bias=attc[:, sb : sb + 1],
                                    scale=SCALE,
                                )
                        else:
                            pqk3 = pqk[:].rearrange("p (b n) -> p b n", b=2)
                            nc.scalar.activation(
                                te3[:, :, xlo:TC],
                                pqk3[:, :, xlo:TC],
                                EXP,
                                bias=attc[:, sb : sb + 1],
                                scale=SCALE,
                            )
                        for blk, mi, coff in _diag_actions(sb, tci):
                            x = blk * SB + coff
                            m_ap = bass_mod.AP(
                                tensor=masks[:].tensor,
                                offset=mi * SB + coff,
                                ap=[[2 * SB + 1, SB], [0, 2], [1, SB - coff]],
                            )
                            nc.vector.tensor_mul(
                                te3[:, :, x : blk * SB + SB],
                                te3[:, :, x : blk * SB + SB],
                                m_ap,
                            )
                        tes.append(te)
                        # interleave projection work for PE while ACT runs
                        for f in feed[i * rate : (i + 1) * rate]:
                            f()
                        if not defer_pv:
                            # stagger: emit PV for t-blocks whose last
                            # contributor's exp has had time to complete
                            while emitted < 4 and last_i[emitted] <= i - stag:
                                emit_pv_group(pair, pvh, tes, emitted, tci, n)
                                emitted += 1
                                if tail_split and emitted == 3:
                                    emit_norm(pair, tci, pvh, 0, 3,
                                              act_dma=True)
                    for f in feed[n * rate :]:
                        f()
                    if defer_pv:
                        return tes
                    while emitted < 4:
                        emit_pv_group(pair, pvh, tes, emitted, tci, n)
                        emitted += 1
                        if tail_split and emitted == 3:
                            emit_norm(pair, tci, pvh, 0, 3, act_dma=True)
                    if tail_split:
                        emit_norm(pair, tci, pvh, 3, 4, dma_eng=nc.sync)
                    else:
                        emit_norm(pair, tci, pvh, 0, 4)
                    return tes

                def emit_pv_chunk(pair, tci, tes, tail_split=False):
                    """Deferred PV+norm for a chunk whose QK ran earlier."""
                    n = len(_alive_sbs(tci))
                    pvh = [pvps.tile([SB, 260], F32, tag="pv",
                                     name=f"dpv{h2}") for h2 in range(2)]
                    for tb in range(4):
                        emit_pv_group(pair, pvh, tes, tb, tci, n)
                        if tail_split and tb == 2:
                            emit_norm(pair, tci, pvh, 0, 3, act_dma=True)
                    if tail_split:
                        emit_norm(pair, tci, pvh, 3, 4, dma_eng=nc.sync)
                    else:
                        emit_norm(pair, tci, pvh, 0, 4)

                def emit_special_probs():
                    """t=512 scores, column-major: psp[s, 4g+sb]; one exp."""
                    psp = ppps.tile([SB, TC], F32, tag="pp", name="psp")
                    for g in range(NHC):
                        pair, h2 = g // 2, g % 2
                        qcol = qt[pair][h2 * HD : (h2 + 1) * HD, 512:513]
                        for sb in range(4):
                            nc.tensor.matmul(
                                psp[:, 4 * g + sb : 4 * g + sb + 1],
                                kt[pair][h2 * HD : (h2 + 1) * HD,
                                         sb * SB : (sb + 1) * SB],
                                qcol,
                                start=True, stop=True,
                                skip_group_check=True,
                                tile_position=(h2 * HD, 0),
                            )
                        # j=512 tail term (partition 0)
                        nc.tensor.matmul(
                            psp[0:1, 16 + g : 17 + g],
                            kt[pair][h2 * HD : (h2 + 1) * HD, 512:513],
                            qcol,
                            start=True, stop=True,
                            skip_group_check=True,
                            tile_position=(h2 * HD, 0),
                        )
                    if with_attc:
                        am = bass_mod.AP(
                            tensor=attcs[:].tensor, offset=0,
                            ap=[[NSB, SB], [0, NHC], [1, 4]],
                        )
                        nc.vector.tensor_add(psp[:, 0:16], psp[:, 0:16], am)
                        am5 = bass_mod.AP(
                            tensor=attr_t[:].tensor, offset=512,
                            ap=[[0, 1], [0, NHC]],
                        )
                        nc.vector.tensor_add(
                            psp[0:1, 16:20], psp[0:1, 16:20], am5
                        )
                    nc.scalar.activation(
                        erT[:, 0:20], psp[:, 0:20], EXP, scale=SCALE
                    )

                def emit_special_pv():
                    """Row-major PV from the column-major probs in erT."""
                    pvs = pvps.tile([SB, 260], F32, tag="pv", name="pvs")
                    for g in range(NHC):
                        for sb4 in range(4):
                            nc.tensor.matmul(
                                pvs[0:1, g * (HD + 1) :
                                    (g + 1) * (HD + 1)],
                                erT[:, 4 * g + sb4 : 4 * g + sb4 + 1],
                                vt[sb4][:, g * (HD + 1) :
                                        (g + 1) * (HD + 1)],
                                start=(sb4 == 0),
                                stop=False,
                                skip_group_check=True,
                            )
                        nc.tensor.matmul(
                            pvs[0:1, g * (HD + 1) : (g + 1) * (HD + 1)],
                            erT[0:1, 16 + g : 17 + g],
                            vt[4][0:1, g * (HD + 1) : (g + 1) * (HD + 1)],
                            start=False, stop=True,
                            skip_group_check=True,
                        )
                    zsrc = bass_mod.AP(
                        tensor=pvs[:].tensor, offset=HD,
                        ap=[[260, 1], [HD + 1, NHC], [1, 1]],
                    )
                    nc.vector.reciprocal(rz4[0:1, 0:NHC], zsrc)
                    for g in range(NHC):
                        nc.vector.tensor_scalar_mul(
                            svn[0:1, g * HD : (g + 1) * HD],
                            pvs[0:1, g * (HD + 1) :
                                g * (HD + 1) + HD],
                            rz4[0:1, g : g + 1],
                        )
                    nc.sync.dma_start(out=out_d[512:513, :], in_=svn[:])

                # startup: q00/k00 interleaved k-major so PE rides the DMA
                # pipeline; v projections become late feed (their only
                # consumers, the PVs, are deferred)
                pre_qk = mmps.tile([SB, 2 * TC], F32, tag="mm", name="preqk")
                for k in range(NK):
                    nc.tensor.matmul(
                        pre_qk[:, 0:TC],
                        wt[:, k * WK : k * WK + SB],
                        hst[:, k * S : k * S + TC],
                        start=(k == 0), stop=(k == NKA - 1),
                    )
                    nc.tensor.matmul(
                        pre_qk[:, TC : 2 * TC],
                        wt[:, k * WK + SB : k * WK + 2 * SB],
                        hst[:, k * S : k * S + TC],
                        start=(k == 0), stop=(k == NKA - 1),
                    )
                if with_bias:
                    nc.tensor.matmul(
                        pre_qk[:, 0:TC], w9[:, 0:SB], hst9[:, 0:TC],
                        start=False, stop=True,
                    )
                    nc.tensor.matmul(
                        pre_qk[:, TC : 2 * TC], w9[:, SB : 2 * SB],
                        hst9[:, 0:TC], start=False, stop=True,
                    )
                nc.vector.tensor_copy(qt[0][:, 0:TC], pre_qk[:, 0:TC])
                nc.vector.tensor_copy(kt[0][:, 0:TC], pre_qk[:, TC : 2 * TC])

                # v4 schedule: QK of every chunk as early as deps allow so
                # ACT (exp) saturates from ~8us on; all PV work floats to
                # wherever PE has slack vs ACT, finishing with a pure-PE
                # endgame that runs while ACT drains the last exps.
                te00 = emit_attn(0, 0, feed=thunks_q(0, 1) + thunks_k(0, 1),
                                 defer_pv=True)
                te01 = emit_attn(0, 1, feed=thunks_q(0, 2), defer_pv=True)
                te02 = emit_attn(0, 2, feed=thunks_q(1, 0) + thunks_k(1, 0),
                                 defer_pv=True)
                te10 = emit_attn(1, 0, feed=thunks_q(1, 1) + thunks_k(1, 1),
                                 defer_pv=True)
                te11 = emit_attn(1, 1, feed=thunks_q(0, 3) + thunks_k(0, 2),
                                 defer_pv=True)
                emit_special_probs()
                te03 = emit_attn(
                    0, 3,
                    feed=thunks_q(1, 2) + thunks_v(0) + thunks_v(1)
                    + thunks_v(2) + thunks_v(3)
                    + [lambda: emit_pv_chunk(0, 0, te00),
                       lambda: emit_pv_chunk(0, 1, te01),
                       lambda: emit_pv_chunk(1, 0, te10)],
                    defer_pv=True)
                te12 = emit_attn(
                    1, 2,
                    feed=thunks_q(1, 3) + thunks_k(1, 2)
                    + [lambda: emit_pv_chunk(1, 1, te11)],
                    defer_pv=True)
                te13 = emit_attn(
                    1, 3,
                    feed=thunks_v(4) + thunks_v(5) + thunks_v(6)
                    + thunks_v(7) + thunks_v(8) + thunks_v(9)
                    + thunks_v(10) + thunks_v(11),
                    defer_pv=True)
                # endgame: pure PE while ACT finishes the (1,3) exps
                emit_pv_chunk(0, 2, te02)
                emit_special_pv()
                emit_pv_chunk(1, 2, te12)
                emit_pv_chunk(0, 3, te03)
                emit_pv_chunk(1, 3, te13, tail_split=True)

    nc.compile()
    return nc


def _host_prep(inputs, with_bias, with_attc):
    import ml_dtypes

    bf = ml_dtypes.bfloat16
    hs = np.asarray(inputs["hidden_states"], dtype=np.float32)
    am = np.asarray(inputs["attention_mask"], dtype=np.float32)
    Wq = np.asarray(inputs["Wq"], dtype=np.float32)
    bq = np.asarray(inputs["bq"], dtype=np.float32)
    Wk = np.asarray(inputs["Wk"], dtype=np.float32)
    bk = np.asarray(inputs["bk"], dtype=np.float32)
    Wv = np.asarray(inputs["Wv"], dtype=np.float32)
    bv = np.asarray(inputs["bv"], dtype=np.float32)

    p = np.arange(SB)[:, None]
    x = np.arange(SB)[None, :]
    m0 = (p <= x).astype(np.float32)
    m1 = (p <= x - 1).astype(np.float32)
    ones = np.ones((SB, 1), dtype=np.float32)
    masks = np.concatenate([m0, m1, ones], axis=1).astype(bf)

    in_maps = []
    for c in range(8):
        b, g = c // 4, c % 4
        # hst [128, 8k*2048]: hst[p, 2048k+t] = hs[b, t, 128k+p]
        hsT = hs[b].T  # [1024, 2048]
        hstp = hsT.reshape(NK, SB, S).transpose(1, 0, 2).reshape(SB, NK * S)
        # w [128, 8k*768]: [q0|k0|q1|k1|v], w[p, 768k+256e+j]=Wq[256g+128e+j,128k+p]
        w = np.zeros((SB, NK, WK), dtype=np.float32)
        Wq_sl = Wq[256 * g : 256 * (g + 1), :]  # [256, 1024]
        Wk_sl = Wk[256 * g : 256 * (g + 1), :]
        Wv_sl = Wv[256 * g : 256 * (g + 1), :]
        for k in range(NK):
            cols = slice(SB * k, SB * (k + 1))
            w[:, k, 0:128] = Wq_sl[0:128, cols].T
            w[:, k, 128:256] = Wk_sl[0:128, cols].T
            w[:, k, 256:384] = Wq_sl[128:256, cols].T
            w[:, k, 384:512] = Wk_sl[128:256, cols].T
            w[:, k, 512:768] = Wv_sl[:, cols].T
        amv = am[b, 0, 0, :].astype(np.float32)
        attc = np.ascontiguousarray(amv.reshape(NSB, SB).T)
        m = {
            "hst": hstp.astype(bf),
            "w": w.reshape(SB, NK * WK).astype(bf),
            "attc": attc,
            "masks": masks.copy(),
            "attr": (amv[:513] / SCALE).reshape(1, 513).copy(),
        }
        if with_attc:
            m["attcs"] = (attc / SCALE).copy()
        if with_bias:
            bsl = np.zeros((WK,), dtype=np.float32)
            bsl[0:128] = bq[256 * g : 256 * g + 128]
            bsl[128:256] = bk[256 * g : 256 * g + 128]
            bsl[256:384] = bq[256 * g + 128 : 256 * g + 256]
            bsl[384:512] = bk[256 * g + 128 : 256 * g + 256]
            bsl[512:768] = bv[256 * g : 256 * (g + 1)]
            m["hst9"] = np.ones((1, S), dtype=np.float32).astype(bf)
            m["w9"] = bsl.reshape(1, WK).astype(bf)
        in_maps.append(m)
    return in_maps


LAST_EXEC_NS = None


def kernel(**inputs):
    import os

    from concourse.bass_utils import run_bass_kernel_spmd

    global LAST_EXEC_NS
    with_bias = bool(
        np.any(np.asarray(inputs["bq"]))
        or np.any(np.asarray(inputs["bk"]))
        or np.any(np.asarray(inputs["bv"]))
    )
    with_attc = bool(np.any(np.asarray(inputs["attention_mask"])))
    key = f"nc{int(with_bias)}{int(with_attc)}"
    if key not in _CACHE:
        _CACHE[key] = _build_program(with_bias=with_bias,
                                     with_attc=with_attc)
    nc = _CACHE[key]
    in_maps = _host_prep(inputs, with_bias, with_attc)
    trace = bool(os.environ.get("BASS_KERNEL_TRACE"))
    res = run_bass_kernel_spmd(nc, in_maps, list(range(8)), trace=trace)
    LAST_EXEC_NS = res.exec_time_ns
    out = np.empty((B, S, H), dtype=np.float32)
    for c in range(8):
        b, g = c // 4, c % 4
        out[b, :, 256 * g : 256 * (g + 1)] = res.results[c]["out"]
    return out

